# Initial kernel scaffold
#
"""Trainium2 Bass kernel: 2D Chebyshev-collocation Helmholtz solver via
fast diagonalization.

The reference solves (Iy kron Dx2 + Dy2 kron Ix - alpha I) u = f with
Dirichlet rows overwritten (boundary values from bc_*). The operator is
separable, so instead of a dense 4096x4096 LU we solve the equivalent
Sylvester form on the 62x62 interior:

    Ay V + V Ax^T - alpha V = G,   Ax/Ay = interior blocks of Dx2/Dy2

With eigendecompositions Ax = Sx Lx Sx^-1, Ay = Sy Ly Sy^-1 (tiny 62x62
solves done on host in fp64), the device work is a chain of four small
matmuls plus one elementwise scale:

    H  = Ty @ Bt @ Tx          (boundary lift folded into Ty/Tx/Bt)
    W  = H * C,  C = 1/(ly_i + lx_j - alpha)
    U  = Ry @ W @ Rx + Z       (embeds interior, Z carries the bc ring)

All device matmuls are arranged so no on-device transpose is needed
(PE computes lhsT.T @ rhs). The 8 NeuronCores run the solve replicated
(data-parallel batch of size 1); core 0's output is returned.
"""

import numpy as np

import concourse.bacc as bacc
import concourse.mybir as mybir
import concourse.tile as tile
from concourse.bass_utils import run_bass_kernel_spmd

N = 64          # grid points per dimension (NX+1 == NY+1 == 64)
M = N - 2       # interior points per dimension
N_CORES = 8
F32 = mybir.dt.float32

_SHAPES = {
    "bt": [N, N],    # B-tilde: rhs grid, bc ring, corners zeroed
    "tyt": [N, M],   # Ty^T,  Ty = Sy^-1 @ Ey
    "tx": [N, M],    # Tx = Ex^T @ Sx^-T
    "ct": [M, M],    # C^T
    "rx": [M, N],    # SxT embedded in cols 1..62
    "ryt": [M, N],   # SyT embedded in cols 1..62
    "z": [N, N],     # boundary frame (interior zero)
}

_CACHE = {}


def _build_nc():
    nc = bacc.Bacc("TRN2", target_bir_lowering=False, debug=False,
                   num_devices=N_CORES)
    ins = {name: nc.dram_tensor(name, shp, F32, kind="ExternalInput").ap()
           for name, shp in _SHAPES.items()}
    out = nc.dram_tensor("out", [N, N], F32, kind="ExternalOutput").ap()

    with tile.TileContext(nc) as tc:
        with (
            tc.tile_pool(name="sbuf", bufs=1) as pool,
            tc.tile_pool(name="psum", bufs=1, space="PSUM") as psum,
        ):
            sb = {}
            for name, shp in _SHAPES.items():
                t = pool.tile(shp, F32, tag=name)
                nc.sync.dma_start(t[:], ins[name][:])
                sb[name] = t

            # m1 = Bt^T @ Ty^T                               [N, M]
            p1 = psum.tile([N, M], F32, tag="p1")
            nc.tensor.matmul(p1[:], sb["bt"][:], sb["tyt"][:],
                             start=True, stop=True)
            m1s = pool.tile([N, M], F32, tag="m1s")
            nc.vector.tensor_copy(m1s[:], p1[:])

            # H^T = Tx^T @ m1                                [M, M]
            p2 = psum.tile([M, M], F32, tag="p2")
            nc.tensor.matmul(p2[:], sb["tx"][:], m1s[:],
                             start=True, stop=True)

            # W^T = H^T * C^T                                [M, M]
            wt = pool.tile([M, M], F32, tag="wt")
            nc.vector.tensor_mul(wt[:], p2[:], sb["ct"][:])

            # m2 = W @ Rx                                    [M, N]
            p3 = psum.tile([M, N], F32, tag="p3")
            nc.tensor.matmul(p3[:], wt[:], sb["rx"][:],
                             start=True, stop=True)
            m2s = pool.tile([M, N], F32, tag="m2s")
            nc.vector.tensor_copy(m2s[:], p3[:])

            # V_full = Ry @ m2  (boundary rows/cols zero)    [N, N]
            p4 = psum.tile([N, N], F32, tag="p4")
            nc.tensor.matmul(p4[:], sb["ryt"][:], m2s[:],
                             start=True, stop=True)

            # U = V_full + Z
            u = pool.tile([N, N], F32, tag="u")
            nc.vector.tensor_add(u[:], p4[:], sb["z"][:])
            nc.sync.dma_start(out[:], u[:])

    nc.compile()
    return nc


def _host_constants(Dx2, Dy2, alpha):
    """fp64 eigen-precompute -> fp32 device operands."""
    Dx2 = np.asarray(Dx2, np.float64)
    Dy2 = np.asarray(Dy2, np.float64)
    alpha = float(alpha)

    Ax = Dx2[1:-1, 1:-1]
    Ay = Dy2[1:-1, 1:-1]
    lamx, Sx = np.linalg.eig(Ax)
    lamy, Sy = np.linalg.eig(Ay)
    lamx = lamx.real; Sx = Sx.real
    lamy = lamy.real; Sy = Sy.real
    Syi = np.linalg.inv(Sy)
    Sxi = np.linalg.inv(Sx)

    # G = Ey @ Bt @ Ex^T pulls the known boundary values to the rhs
    # (valid because Bt's corners are zeroed).
    Ey = np.zeros((M, N)); Ey[:, 1:-1] = np.eye(M)
    Ey[:, 0] = -Dy2[1:-1, 0]; Ey[:, -1] = -Dy2[1:-1, -1]
    Ex = np.zeros((M, N)); Ex[:, 1:-1] = np.eye(M)
    Ex[:, 0] = -Dx2[1:-1, 0]; Ex[:, -1] = -Dx2[1:-1, -1]

    Ty = Syi @ Ey
    Tx = Ex.T @ Sxi.T
    C = 1.0 / (lamy[:, None] + lamx[None, :] - alpha)

    rx = np.zeros((M, N), np.float32); rx[:, 1:-1] = Sx.T
    ryt = np.zeros((M, N), np.float32); ryt[:, 1:-1] = Sy.T
    return {
        "tyt": np.ascontiguousarray(Ty.T, dtype=np.float32),
        "tx": np.ascontiguousarray(Tx, dtype=np.float32),
        "ct": np.ascontiguousarray(C.T, dtype=np.float32),
        "rx": rx,
        "ryt": ryt,
    }


def _pack_rhs(f, bc_top, bc_bottom, bc_left, bc_right):
    f = np.asarray(f, np.float32)
    Bt = f.copy()
    # reference orientation: col 0 <- bc_right, col -1 <- bc_left;
    # column writes come last so they win the corners (as in reference)
    Bt[0, :] = bc_top; Bt[-1, :] = bc_bottom
    Bt[:, 0] = bc_right; Bt[:, -1] = bc_left
    Z = Bt.copy(); Z[1:-1, 1:-1] = 0.0
    Bt[0, 0] = Bt[0, -1] = Bt[-1, 0] = Bt[-1, -1] = 0.0
    return np.ascontiguousarray(Bt), np.ascontiguousarray(Z)


def kernel(f, alpha, bc_top, bc_bottom, bc_left, bc_right, Dx2, Dy2):
    nc = _CACHE.get("nc")
    if nc is None:
        nc = _build_nc()
        _CACHE["nc"] = nc

    K = _host_constants(Dx2, Dy2, alpha)
    Bt, Z = _pack_rhs(f, bc_top, bc_bottom, bc_left, bc_right)
    in_map = {"bt": Bt, "z": Z, **K}
    in_maps = [dict(in_map) for _ in range(N_CORES)]

    res = run_bass_kernel_spmd(nc, in_maps, list(range(N_CORES)))
    return np.asarray(res.results[0]["out"], dtype=np.float32)


# revision 22
# speedup vs baseline: 1.2515x; 1.2515x over previous
"""Trainium2 Bass kernel: 2D Chebyshev-collocation Helmholtz solver via
fast diagonalization.

The reference solves (Iy kron Dx2 + Dy2 kron Ix - alpha I) u = f with
Dirichlet rows overwritten (boundary values from bc_*). The operator is
separable, so instead of a dense 4096x4096 LU we solve the equivalent
Sylvester form on the 62x62 interior:

    Ay V + V Ax^T - alpha V = G,   Ax/Ay = interior blocks of Dx2/Dy2

With eigendecompositions Ax = Sx Lx Sx^-1, Ay = Sy Ly Sy^-1 (tiny 62x62
solves done on host in fp64), the device work is a chain of four small
matmuls plus one elementwise scale:

    H  = Ty @ Bt @ Tx          (boundary lift folded into Ty/Tx/Bt)
    W  = H * C,  C = 1/(ly_i + lx_j - alpha)
    U  = Ry @ W @ Rx + Z       (embeds interior, Z carries the bc ring)

All matmuls are arranged so no on-device transpose is needed (PE
computes lhsT.T @ rhs); operands are fed as float32r (single-pass fp32
matmul). Raw Bass, no Tile framework and no Block/semaphore teardown
barriers: one split input DMA, a manually-semaphored chain on
Sync/PE/DVE, one output DMA. The 8 NeuronCores run the solve replicated
(data-parallel batch of size 1); core 0's output is returned.
"""

import numpy as np

import concourse.bacc as bacc
import concourse.mybir as mybir
from concourse.bass_utils import run_bass_kernel_spmd

N = 64          # grid points per dimension (NX+1 == NY+1 == 64)
M = N - 2       # interior points per dimension
N_CORES = 8
F32 = mybir.dt.float32
F32R = mybir.dt.float32r

# column offsets of the operands inside the packed [64, BLOB_W] input;
# cols 0:250 (bt+tyt+tx+ct — everything mm1/mm2/mul need) ship in DMA1 so
# the front of the chain is never gated on DMA2
_OFF = {
    "bt": 0,      # [N, N]  B-tilde: rhs grid, bc ring, corners zeroed
    "tyt": 64,    # [N, M]  Ty^T,  Ty = Sy^-1 @ Ey
    "tx": 126,    # [N, M]  Tx = Ex^T @ Sx^-T
    "ct": 188,    # [M, M]  C^T
    "rx": 250,    # [M, N]  Sx^T embedded in cols 1..62
    "ryt": 314,   # [M, N]  Sy^T embedded in cols 1..62
    "z": 378,     # [N, N]  boundary frame (interior zero)
}
_SPLIT = 250
BLOB_W = 448

_CACHE = {}


def _build_nc():
    nc = bacc.Bacc("TRN2", target_bir_lowering=False, debug=False,
                   num_devices=N_CORES)
    blob_d = nc.dram_tensor("blob", [N, BLOB_W], F32R, kind="ExternalInput").ap()
    out_d = nc.dram_tensor("out", [N, N], F32, kind="ExternalOutput").ap()

    blob = nc.alloc_sbuf_tensor("blob_sb", [N, BLOB_W], F32R)
    m1s = nc.alloc_sbuf_tensor("m1s", [N, M], F32R)
    wt = nc.alloc_sbuf_tensor("wt", [M, M], F32R)
    m2s = nc.alloc_sbuf_tensor("m2s", [M, N], F32R)
    u = nc.alloc_sbuf_tensor("u", [N, N], F32)
    p1 = nc.alloc_psum_tensor("p1", [N, M], F32)
    p2 = nc.alloc_psum_tensor("p2", [M, M], F32)
    p3 = nc.alloc_psum_tensor("p3", [M, N], F32)
    p4 = nc.alloc_psum_tensor("p4", [N, N], F32)

    def op(name, cast=None):
        c0 = _OFF[name]
        rows = N if name in ("bt", "tyt", "tx", "z") else M
        cols = {"bt": N, "tyt": M, "tx": M, "ct": M, "rx": N, "ryt": N,
                "z": N}[name]
        ap = blob.ap()[0:rows, c0:c0 + cols]
        return ap.bitcast(cast) if cast else ap

    dsem1 = nc.alloc_semaphore("dsem1")
    dsem2 = nc.alloc_semaphore("dsem2")
    dsem3 = nc.alloc_semaphore("dsem3")
    dsem4 = nc.alloc_semaphore("dsem4")
    tsem = nc.alloc_semaphore("tsem")
    vsem = nc.alloc_semaphore("vsem")

    # ---- Sync engine: DMAs ----
    # inputs grouped by the chain stage that first needs them
    in_dma1 = nc.sync.dma_start(out=blob.ap()[:, 0:BLOB_W],
                                in_=blob_d[:, 0:BLOB_W]).then_inc(dsem1, 16)
    nc.sync.wait_ge(vsem, 4)
    nc.sync.dma_start(out=out_d[:, :], in_=u.ap()[:, :]).then_inc(dsem4, 16)
    nc.sync.wait_ge(dsem4, 16)   # output landed in DRAM before program end

    # ---- Tensor engine: 4 chained matmuls (fp32r operands) ----
    nc.tensor.wait_ge(dsem1, 16)
    # m1 = Bt^T @ Ty^T                             [N, M]
    nc.tensor.matmul(p1.ap()[:, :], op("bt"), op("tyt"),
                     start=True, stop=True).then_inc(tsem, 1)
    nc.tensor.wait_ge(vsem, 1)
    # H^T = Tx^T @ m1                              [M, M]
    nc.tensor.matmul(p2.ap()[:, :], op("tx"), m1s.ap()[:, :],
                     start=True, stop=True).then_inc(tsem, 1)
    nc.tensor.wait_ge(vsem, 2)
    # m2 = W @ Rx                                  [M, N]
    nc.tensor.matmul(p3.ap()[:, :], wt.ap()[:, :], op("rx"),
                     start=True, stop=True).then_inc(tsem, 1)
    nc.tensor.wait_ge(vsem, 3)
    # V_full = Ry @ m2 (boundary rows/cols zero)   [N, N]
    nc.tensor.matmul(p4.ap()[:, :], op("ryt"), m2s.ap()[:, :],
                     start=True, stop=True).then_inc(tsem, 1)

    # ---- Vector engine: PSUM->SBUF moves + pointwise (f32r-rounded outs) ----
    nc.vector.wait_ge(tsem, 1)
    nc.vector.tensor_copy(m1s.ap()[:, :], p1.ap()[:, :]).then_inc(vsem, 1)
    nc.vector.wait_ge(tsem, 2)
    # W^T = H^T * C^T
    nc.vector.tensor_mul(wt.ap()[:, :], p2.ap()[:, :],
                         op("ct", F32)).then_inc(vsem, 1)
    nc.vector.wait_ge(tsem, 3)
    nc.vector.tensor_copy(m2s.ap()[:, :], p3.ap()[:, :]).then_inc(vsem, 1)
    nc.vector.wait_ge(tsem, 4)
    # U = V_full + Z
    nc.vector.tensor_add(u.ap()[:, :], p4.ap()[:, :],
                         op("z", F32)).then_inc(vsem, 1)

    # Hoist the two input DMA issues to the head of the block so the SP
    # engine triggers them before the framework's init barrier; the input
    # data is already in DRAM when the NEFF starts, and the transfers then
    # complete behind the barrier instead of on the critical path.
    blk = nc.main_func.blocks[0]
    insts = blk.instructions
    dma_names = {in_dma1.ins.name}
    hoisted = [i for i in insts if i.name in dma_names]
    rest = [i for i in insts if i.name not in dma_names]
    insts[:] = rest[:1] + hoisted + rest[1:]   # keep dummycall first

    nc.compile()
    return nc


def _host_constants(Dx2, Dy2, alpha):
    """fp64 eigen-precompute -> fp32 device operands."""
    Dx2 = np.asarray(Dx2, np.float64)
    Dy2 = np.asarray(Dy2, np.float64)
    alpha = float(alpha)

    Ax = Dx2[1:-1, 1:-1]
    Ay = Dy2[1:-1, 1:-1]
    lamx, Sx = np.linalg.eig(Ax)
    lamy, Sy = np.linalg.eig(Ay)
    lamx = lamx.real; Sx = Sx.real
    lamy = lamy.real; Sy = Sy.real
    Syi = np.linalg.inv(Sy)
    Sxi = np.linalg.inv(Sx)

    # G = Ey @ Bt @ Ex^T pulls the known boundary values to the rhs
    # (valid because Bt's corners are zeroed).
    Ey = np.zeros((M, N)); Ey[:, 1:-1] = np.eye(M)
    Ey[:, 0] = -Dy2[1:-1, 0]; Ey[:, -1] = -Dy2[1:-1, -1]
    Ex = np.zeros((M, N)); Ex[:, 1:-1] = np.eye(M)
    Ex[:, 0] = -Dx2[1:-1, 0]; Ex[:, -1] = -Dx2[1:-1, -1]

    Ty = Syi @ Ey
    Tx = Ex.T @ Sxi.T
    C = 1.0 / (lamy[:, None] + lamx[None, :] - alpha)

    K = {
        "tyt": np.ascontiguousarray(Ty.T, dtype=np.float32),
        "tx": np.ascontiguousarray(Tx, dtype=np.float32),
        "ct": np.ascontiguousarray(C.T, dtype=np.float32),
    }
    rx = np.zeros((M, N), np.float32); rx[:, 1:-1] = Sx.T
    ryt = np.zeros((M, N), np.float32); ryt[:, 1:-1] = Sy.T
    K["rx"] = rx
    K["ryt"] = ryt
    return K


def _pack_rhs(f, bc_top, bc_bottom, bc_left, bc_right):
    f = np.asarray(f, np.float32)
    Bt = f.copy()
    # reference orientation: col 0 <- bc_right, col -1 <- bc_left;
    # column writes come last so they win the corners (as in reference)
    Bt[0, :] = bc_top; Bt[-1, :] = bc_bottom
    Bt[:, 0] = bc_right; Bt[:, -1] = bc_left
    Z = Bt.copy(); Z[1:-1, 1:-1] = 0.0
    Bt[0, 0] = Bt[0, -1] = Bt[-1, 0] = Bt[-1, -1] = 0.0
    return Bt, Z


def _pack_blob(f, alpha, bc_top, bc_bottom, bc_left, bc_right, Dx2, Dy2):
    K = _host_constants(Dx2, Dy2, alpha)
    Bt, Z = _pack_rhs(f, bc_top, bc_bottom, bc_left, bc_right)
    blob = np.zeros((N, BLOB_W), np.float32)
    pieces = {"bt": Bt, "z": Z, **K}
    for name, arr in pieces.items():
        r, c = arr.shape
        blob[0:r, _OFF[name]:_OFF[name] + c] = arr
    return blob


def kernel(f, alpha, bc_top, bc_bottom, bc_left, bc_right, Dx2, Dy2):
    nc = _CACHE.get("nc")
    if nc is None:
        nc = _build_nc()
        _CACHE["nc"] = nc

    blob = _pack_blob(f, alpha, bc_top, bc_bottom, bc_left, bc_right, Dx2, Dy2)
    in_maps = [{"blob": blob.copy()} for _ in range(N_CORES)]
    res = run_bass_kernel_spmd(nc, in_maps, list(range(N_CORES)))
    return np.asarray(res.results[0]["out"], dtype=np.float32)
